# revision 7
# baseline (speedup 1.0000x reference)
"""Trainium2 Bass kernel for nn_MoE_27041114095775 (moe_routing).

Expert-parallel MoE across 8 NeuronCores:
  - router + top-4 selection computed on every core (fp32 for exact selection)
  - each core owns 2 experts (gate_up/down sharded on the expert axis)
  - shared-expert FFN tensor-parallel along its intermediate dim (96/core)
  - on-device token compaction (cumsum via triangular matmuls) feeds a
    transposing dma_gather; SwiGLU in bf16 with fp32 PSUM accumulate;
    weighted outputs dma_scatter_add'ed into a [T,H] fp32 accumulator
  - ReduceScatter(add) combines the 8 partial accumulators; each core
    returns a 256-token slice of the final output.

Self-contained: hardcodes all shapes; host side only shards/reformats
inputs and reassembles the output.
"""
import numpy as np
import ml_dtypes
from contextlib import ExitStack

BF16 = ml_dtypes.bfloat16

# ---- problem dims (hardcoded) ----
B, S, H = 1, 2048, 2048
E, I, IS = 16, 768, 768
TOPK = 4
T = B * S
NCORES = 8
EPC = E // NCORES          # experts per core = 2
ISC = IS // NCORES         # shared intermediate per core = 96
CAP = 768                  # per-expert token capacity (multiple of 128)
NT = T // 128              # 16 token tiles
NK = H // 128              # 16 contraction tiles
NTC = CAP // 128           # 6 capacity tiles
NJ = 2 * I // 128          # 12 gate_up row tiles
NI = I // 128              # 6 intermediate tiles
NHC = H // 512             # 4 hidden 512-chunks

_CACHE = {}


def _build():
    import concourse.bass as bass
    import concourse.tile as tile
    from concourse import bacc, mybir, library_config
    from concourse.expressions import smin, smax

    f32 = mybir.dt.float32
    bf16 = mybir.dt.bfloat16
    i16 = mybir.dt.int16
    i32 = mybir.dt.int32
    MM = mybir.AluOpType

    nc = bacc.Bacc("TRN2", target_bir_lowering=False, debug=False,
                   num_devices=NCORES)

    # ---- external inputs ----
    xT_f32 = nc.dram_tensor("xT_f32", [H, T], f32, kind="ExternalInput")
    x_bf16 = nc.dram_tensor("x_bf16", [T, H], bf16, kind="ExternalInput")
    rwT = nc.dram_tensor("rwT", [H, E], f32, kind="ExternalInput")
    ebias = nc.dram_tensor("ebias", [E, 1], f32, kind="ExternalInput")
    guT_r = nc.dram_tensor("guT_r", [EPC, NK, NJ, 128, 128], bf16, kind="ExternalInput")
    dT_r = nc.dram_tensor("dT_r", [EPC, NI, NHC, 128, 512], bf16, kind="ExternalInput")
    sgT = nc.dram_tensor("sgT", [H, ISC], bf16, kind="ExternalInput")
    suT = nc.dram_tensor("suT", [H, ISC], bf16, kind="ExternalInput")
    sdT = nc.dram_tensor("sdT", [ISC, H], bf16, kind="ExternalInput")
    idn = nc.dram_tensor("idn", [128, 128], f32, kind="ExternalInput")
    U128 = nc.dram_tensor("U128", [128, 128], f32, kind="ExternalInput")
    iotaR = nc.dram_tensor("iotaR", [128, 128], f32, kind="ExternalInput")
    tokid = nc.dram_tensor("tokid", [128, NT], f32, kind="ExternalInput")
    iotaW = nc.dram_tensor("iotaW", [16, CAP // 16], f32, kind="ExternalInput")
    msk = nc.dram_tensor("msk", [EPC, 128, E], f32, kind="ExternalInput")

    # ---- outputs ----
    out_p = nc.dram_tensor("out", [T // NCORES, H], f32, kind="ExternalOutput")

    # ---- internal DRAM ----
    acc = nc.dram_tensor("acc", [T, H], f32)
    rs_res = nc.dram_tensor("rs_res", [T // NCORES, H], f32)
    lists_d = nc.dram_tensor("lists_d", [EPC, T + 256], f32)
    gats_d = nc.dram_tensor("gats_d", [EPC, T + 256], f32)
    cnt_d = nc.dram_tensor("cnt_d", [1, EPC], f32)

    with tile.TileContext(nc) as tc:
        with ExitStack() as ctx:
            cpool = ctx.enter_context(tc.tile_pool(name="consts", bufs=1))
            xpool = ctx.enter_context(tc.tile_pool(name="xstream", bufs=6))
            rpool = ctx.enter_context(tc.tile_pool(name="routing", bufs=1))
            tpool = ctx.enter_context(tc.tile_pool(name="topk", bufs=4))
            wpool = ctx.enter_context(tc.tile_pool(name="weights", bufs=4))
            apool = ctx.enter_context(tc.tile_pool(name="acts", bufs=2))
            gpool = ctx.enter_context(tc.tile_pool(name="gath", bufs=1))
            spool = ctx.enter_context(tc.tile_pool(name="stage", bufs=2))
            scpool = ctx.enter_context(tc.tile_pool(name="scst", bufs=4))
            bigpool = ctx.enter_context(tc.tile_pool(name="bigstage", bufs=2))

            nc.gpsimd.load_library(library_config.mlp)

            # ---- constants to SBUF ----
            idn_sb = cpool.tile([128, 128], f32)
            nc.sync.dma_start(idn_sb[:], idn[:])
            U_sb = cpool.tile([128, 128], f32)
            nc.sync.dma_start(U_sb[:], U128[:])
            iotaR_sb = cpool.tile([128, 128], f32)
            nc.sync.dma_start(iotaR_sb[:], iotaR[:])
            tokid_sb = cpool.tile([128, NT], f32)
            nc.sync.dma_start(tokid_sb[:], tokid[:])
            iotaW_sb = cpool.tile([16, CAP // 16], f32)
            nc.sync.dma_start(iotaW_sb[:], iotaW[:])
            ebias_sb = cpool.tile([E, 1], f32)
            nc.sync.dma_start(ebias_sb[:], ebias[:])
            msk_sb = cpool.tile([128, EPC, E], f32)
            nc.sync.dma_start(msk_sb[:], msk[:].rearrange("l p e -> p l e"))
            rw_sb = cpool.tile([128, NK, E], f32)
            nc.sync.dma_start(rw_sb[:], rwT[:].rearrange("(k p) e -> p k e", p=128))
            sg_sb = cpool.tile([128, NK, ISC], bf16)
            nc.sync.dma_start(sg_sb[:], sgT[:].rearrange("(k p) i -> p k i", p=128))
            su_sb = cpool.tile([128, NK, ISC], bf16)
            nc.sync.dma_start(su_sb[:], suT[:].rearrange("(k p) i -> p k i", p=128))
            sd_sb = cpool.tile([ISC, H], bf16)
            nc.sync.dma_start(sd_sb[:], sdT[:])

            # ---- phase 1: router (fp32) + shared gate/up (bf16), one pass over xT ----
            scT = rpool.tile([E, T], f32)       # sigmoid scores, [e, t]
            bT = rpool.tile([E, T], f32)        # biased scores, [e, t]
            s_act = rpool.tile([ISC, T], bf16)  # shared silu(g)*u, [i_s, t]
            pA_cm = tc.tile_pool(name="pA", bufs=2, space="PSUM"); pA = pA_cm.__enter__()
            for tcn in range(4):
                t0 = 512 * tcn
                ps_r = pA.tile([E, 512], f32, tag="ps_r")
                ps_g = pA.tile([ISC, 512], f32, tag="ps_g")
                ps_u = pA.tile([ISC, 512], f32, tag="ps_u")
                for k in range(NK):
                    xt = xpool.tile([128, 512], f32, tag="xt")
                    nc.sync.dma_start(xt[:], xT_f32[128 * k:128 * (k + 1), t0:t0 + 512])
                    xtb = xpool.tile([128, 512], bf16, tag="xtb")
                    nc.vector.tensor_copy(xtb[:], xt[:])
                    nc.tensor.matmul(ps_r[:], rw_sb[:, k, :], xt[:],
                                     start=(k == 0), stop=(k == NK - 1))
                    nc.tensor.matmul(ps_g[:], sg_sb[:, k, :], xtb[:],
                                     start=(k == 0), stop=(k == NK - 1))
                    nc.tensor.matmul(ps_u[:], su_sb[:, k, :], xtb[:],
                                     start=(k == 0), stop=(k == NK - 1))
                nc.scalar.activation(scT[:, t0:t0 + 512], ps_r[:],
                                     mybir.ActivationFunctionType.Sigmoid)
                nc.vector.tensor_scalar(out=bT[:, t0:t0 + 512], in0=scT[:, t0:t0 + 512],
                                        scalar1=ebias_sb[:, 0:1], scalar2=None,
                                        op0=MM.add)
                st = spool.tile([ISC, 512], f32, tag="silu_s")
                nc.scalar.activation(st[:], ps_g[:],
                                     mybir.ActivationFunctionType.Silu)
                nc.vector.tensor_tensor(out=s_act[:, t0:t0 + 512], in0=st[:],
                                        in1=ps_u[:], op=MM.mult)

            # ---- phase 2: per-tile top-k, combine weights, compaction ----
            pA_cm.__exit__(None, None, None)
            pO_cm = tc.tile_pool(name="pO", bufs=1, space="PSUM"); pO = pO_cm.__enter__()
            pB2_cm = tc.tile_pool(name="pB2", bufs=2, space="PSUM"); pB2 = pB2_cm.__enter__()
            pB1_cm = tc.tile_pool(name="pB1", bufs=1, space="PSUM"); pB1 = pB1_cm.__enter__()
            ps_off = pO.tile([1, EPC], f32)  # running token counts (excl prefix)
            for j in range(NT):
                c0 = 128 * j
                tp_s = pB2.tile([128, E], f32, tag="tp_s")
                nc.tensor.transpose(tp_s[:], scT[:, c0:c0 + 128], idn_sb[0:E, 0:E])
                tp_b = pB2.tile([128, E], f32, tag="tp_b")
                nc.tensor.transpose(tp_b[:], bT[:, c0:c0 + 128], idn_sb[0:E, 0:E])
                sc_t = tpool.tile([128, E], f32, tag="sc_t")
                nc.vector.tensor_copy(sc_t[:], tp_s[:])
                b_t = tpool.tile([128, E], f32, tag="b_t")
                nc.vector.tensor_copy(b_t[:], tp_b[:])
                mx8 = tpool.tile([128, 8], f32, tag="mx8")
                nc.vector.max(out=mx8[:], in_=b_t[:])
                sel = tpool.tile([128, E], f32, tag="sel")
                nc.vector.tensor_scalar(out=sel[:], in0=b_t[:],
                                        scalar1=mx8[:, TOPK - 1:TOPK], scalar2=None,
                                        op0=MM.is_ge)
                w = tpool.tile([128, E], f32, tag="w")
                nc.vector.tensor_tensor(out=w[:], in0=sc_t[:], in1=sel[:], op=MM.mult)
                sums = tpool.tile([128, 1], f32, tag="sums")
                nc.vector.tensor_reduce(out=sums[:], in_=w[:],
                                        axis=mybir.AxisListType.X, op=MM.add)
                den = tpool.tile([128, 1], f32, tag="den")
                nc.vector.tensor_scalar(out=den[:], in0=sums[:], scalar1=1e-20,
                                        scalar2=None, op0=MM.add)
                rcp = tpool.tile([128, 1], f32, tag="rcp")
                nc.vector.reciprocal(rcp[:], den[:])
                comb = tpool.tile([128, E], f32, tag="comb")
                nc.vector.tensor_scalar(out=comb[:], in0=w[:], scalar1=rcp[:, 0:1],
                                        scalar2=None, op0=MM.mult)

                sel01 = tpool.tile([128, EPC], f32, tag="sel01")
                sts = []
                for l in range(EPC):
                    t16 = tpool.tile([128, E], f32, tag="t16")
                    nc.vector.tensor_tensor(out=t16[:], in0=sel[:],
                                            in1=msk_sb[:, l, :], op=MM.mult)
                    nc.vector.tensor_reduce(out=sel01[:, l:l + 1], in_=t16[:],
                                            axis=mybir.AxisListType.X, op=MM.add)
                    st_l = tpool.tile([128, 2], f32, tag=f"st{l}")
                    nc.vector.tensor_copy(st_l[:, 0:1], tokid_sb[:, j:j + 1])
                    t16b = tpool.tile([128, E], f32, tag="t16b")
                    nc.vector.tensor_tensor(out=t16b[:], in0=comb[:],
                                            in1=msk_sb[:, l, :], op=MM.mult)
                    nc.vector.tensor_reduce(out=st_l[:, 1:2], in_=t16b[:],
                                            axis=mybir.AxisListType.X, op=MM.add)
                    sts.append(st_l)

                # local inclusive cumsum -> local exclusive positions
                ps_pos = pB1.tile([128, EPC], f32, tag="ps_pos")
                nc.tensor.matmul(ps_pos[:], U_sb[:], sel01[:], start=True, stop=True)
                pos01 = tpool.tile([128, EPC], f32, tag="pos01")
                nc.vector.tensor_tensor(out=pos01[:], in0=ps_pos[:], in1=sel01[:],
                                        op=MM.subtract)

                # snapshot running offsets BEFORE adding this tile's totals
                off_j = tpool.tile([1, EPC], f32, tag="off_f")
                if j == 0:
                    nc.vector.memset(off_j[:], 0.0)
                else:
                    nc.vector.tensor_copy(off_j[:], ps_off[:])
                off_i = tpool.tile([1, EPC], i32, tag="off_i")
                nc.vector.tensor_copy(off_i[:], off_j[:])
                nc.tensor.matmul(ps_off[:], U_sb[:, 127:128], sel01[:],
                                 start=(j == 0), stop=(j == NT - 1))

                for l in range(EPC):
                    O_l = tpool.tile([128, 128], f32, tag="O_l")
                    nc.vector.tensor_scalar(out=O_l[:], in0=iotaR_sb[:],
                                            scalar1=pos01[:, l:l + 1],
                                            scalar2=sel01[:, l:l + 1],
                                            op0=MM.is_equal, op1=MM.mult)
                    ps_seg = pB1.tile([128, 2], f32, tag="ps_seg")
                    nc.tensor.matmul(ps_seg[:], O_l[:], sts[l][:], start=True, stop=True)
                    seg = tpool.tile([128, 2], f32, tag="seg")
                    nc.vector.tensor_copy(seg[:], ps_seg[:])
                    offv = nc.sync.value_load(off_i[:, l:l + 1])
                    nc.sync.dma_start(lists_d[l, bass.ds(offv, 128)], seg[:, 0:1])
                    offv2 = nc.sync.value_load(off_i[:, l:l + 1])
                    nc.sync.dma_start(gats_d[l, bass.ds(offv2, 128)], seg[:, 1:2])

            # final counts
            cnt_f = rpool.tile([1, EPC], f32)
            nc.vector.tensor_copy(cnt_f[:], ps_off[:])
            cnt_i = rpool.tile([1, EPC], i32)
            nc.vector.tensor_copy(cnt_i[:], cnt_f[:])
            nc.sync.dma_start(cnt_d[:], cnt_f[:])

            # ---- phase 3: shared-expert down projection -> initializes acc ----
            pB1_cm.__exit__(None, None, None)
            pB2_cm.__exit__(None, None, None)
            pO_cm.__exit__(None, None, None)
            pC_gu_cm = tc.tile_pool(name="pC_gu", bufs=2, space="PSUM"); pC_gu = pC_gu_cm.__enter__()
            pC_o_cm = tc.tile_pool(name="pC_o", bufs=1, space="PSUM"); pC_o = pC_o_cm.__enter__()
            pC_y_cm = tc.tile_pool(name="pC_y", bufs=1, space="PSUM"); pC_y = pC_y_cm.__enter__()
            for tt in range(NT):
                sto = bigpool.tile([128, H], f32, tag="sto")
                for hc in range(NHC):
                    ps_o = pC_o.tile([128, 512], f32, tag="ps_o")
                    nc.tensor.matmul(ps_o[:], s_act[:, 128 * tt:128 * (tt + 1)],
                                     sd_sb[:, 512 * hc:512 * (hc + 1)],
                                     start=True, stop=True)
                    nc.vector.tensor_copy(sto[:, 512 * hc:512 * (hc + 1)], ps_o[:])
                nc.sync.dma_start(acc[128 * tt:128 * (tt + 1), :], sto[:])

            # ---- phase 4: per-expert index prep, gather, FFN, scatter ----
            for l in range(EPC):
                # counts -> registers (gpsimd for gather/scatter, clamped to CAP)
                cntv = nc.gpsimd.value_load(cnt_i[:, l:l + 1])
                cntc = smin(cntv, CAP)
                # broadcast count to 16 partitions for sanitize
                cnt16 = tpool.tile([16, 1], f32, tag="cnt16")
                nc.sync.dma_start(cnt16[:], cnt_d[0:1, l:l + 1].to_broadcast([16, 1]))

                # sanitize list: positions >= count -> -1 ; wrapped [16, CAP/16]
                lw = tpool.tile([16, CAP // 16], f32, tag="lw")
                nc.sync.dma_start(
                    lw[:], lists_d[l, 0:CAP].rearrange("(f p) -> p f", p=16))
                m = tpool.tile([16, CAP // 16], f32, tag="m")
                nc.vector.tensor_scalar(out=m[:], in0=iotaW_sb[:],
                                        scalar1=cnt16[:, 0:1], scalar2=None,
                                        op0=MM.is_lt)
                t1 = tpool.tile([16, CAP // 16], f32, tag="t1")
                nc.vector.tensor_scalar(out=t1[:], in0=lw[:], scalar1=1.0,
                                        scalar2=None, op0=MM.add)
                nc.vector.tensor_tensor(out=t1[:], in0=t1[:], in1=m[:], op=MM.mult)
                nc.vector.tensor_scalar(out=t1[:], in0=t1[:], scalar1=1.0,
                                        scalar2=None, op0=MM.subtract)
                li = tpool.tile([16, CAP // 16], i16, tag="li")
                nc.vector.tensor_copy(li[:], t1[:])
                idx_l = gpool.tile([128, CAP // 16], i16, tag=f"idx{l}")
                nc.sync.dma_start(idx_l[0:16, :], li[:])
                nc.sync.dma_start(idx_l[16:32, :], idx_l[0:16, :])
                nc.sync.dma_start(idx_l[32:64, :], idx_l[0:32, :])
                nc.sync.dma_start(idx_l[64:128, :], idx_l[0:64, :])
                gat_l = gpool.tile([128, NTC], f32, tag=f"gat{l}")
                nc.sync.dma_start(
                    gat_l[:], gats_d[l, 0:CAP].rearrange("(f p) -> p f", p=128))

                # gather tokens, transposed: xg[p, k, i] = x[list[i], 128k+p]
                xg = gpool.tile([128, NK, CAP], bf16, tag=f"xg{l}")
                nc.gpsimd.dma_gather(
                    out_ap=xg[:], in_ap=x_bf16[:], idxs_ap=idx_l[:],
                    num_idxs=CAP, num_idxs_reg=cntc, elem_size=H, transpose=True)

                # gate_up + SwiGLU
                act_l = apool.tile([128, NI, CAP], bf16, tag="act")
                for pp in range(NI):
                    ps_pair = []
                    for jj in (pp, pp + NI):
                        ps = pC_gu.tile([128, CAP], f32, tag="ps_gu")
                        for k in range(NK):
                            wt = wpool.tile([128, 128], bf16, tag="wt_gu")
                            nc.sync.dma_start(wt[:], guT_r[l, k, jj, :, :])
                            nc.tensor.matmul(ps[:, 0:512], wt[:], xg[:, k, 0:512],
                                             start=(k == 0), stop=(k == NK - 1))
                            nc.tensor.matmul(ps[:, 512:CAP], wt[:], xg[:, k, 512:CAP],
                                             start=(k == 0), stop=(k == NK - 1))
                        ps_pair.append(ps)
                    silu_t = spool.tile([128, CAP], f32, tag="silu_t")
                    nc.scalar.activation(silu_t[:], ps_pair[0][:],
                                         mybir.ActivationFunctionType.Silu)
                    nc.vector.tensor_tensor(out=act_l[:, pp, :], in0=silu_t[:],
                                            in1=ps_pair[1][:], op=MM.mult)

                # down projection + gating scale + scatter-add into acc
                for half in range(2):
                    tts = [3 * half, 3 * half + 1, 3 * half + 2]
                    for hc in range(NHC):
                        ps_y = [pC_y.tile([128, 512], f32, tag=f"ps_y{q}",
                                          name=f"ps_y{q}_{l}_{half}_{hc}")
                                for q in range(3)]
                        for it in range(NI):
                            dwt = wpool.tile([128, 512], bf16, tag="wt_d")
                            nc.sync.dma_start(dwt[:], dT_r[l, it, hc, :, :])
                            for q, tt in enumerate(tts):
                                nc.tensor.matmul(
                                    ps_y[q][:],
                                    act_l[:, it, 128 * tt:128 * (tt + 1)], dwt[:],
                                    start=(it == 0), stop=(it == NI - 1))
                        for q, tt in enumerate(tts):
                            sc = scpool.tile([128, 1, 512], f32, tag="sc")
                            nc.vector.tensor_scalar(out=sc[:, 0, :], in0=ps_y[q][:],
                                                    scalar1=gat_l[:, tt:tt + 1],
                                                    scalar2=None, op0=MM.mult)
                            ch = smin(smax(cntv - 128 * tt, 0), 128)
                            nc.gpsimd.dma_scatter_add(
                                out_ap=acc[:, 512 * hc:512 * (hc + 1)],
                                in_ap=sc[:], idxs_ap=idx_l[:, 8 * tt:8 * (tt + 1)],
                                num_idxs=128, num_idxs_reg=ch,
                                elem_size=512, elem_step=H)

            # ---- phase 5: ReduceScatter + output ----
            pC_y_cm.__exit__(None, None, None)
            pC_o_cm.__exit__(None, None, None)
            pC_gu_cm.__exit__(None, None, None)
            nc.gpsimd.collective_compute(
                "ReduceScatter", mybir.AluOpType.add,
                replica_groups=[list(range(NCORES))],
                ins=[acc[:]], outs=[rs_res[:]])
            for i in range(2):
                ot = bigpool.tile([128, H], f32, tag="sto", name=f"ot{i}")
                nc.sync.dma_start(ot[:], rs_res[128 * i:128 * (i + 1), :])
                nc.sync.dma_start(out_p[128 * i:128 * (i + 1), :], ot[:])

    nc.compile()
    return nc


def _host_prep(inputs):
    """Build the 8 per-core input maps from full inputs."""
    x = np.ascontiguousarray(inputs["hidden_states"].reshape(T, H), np.float32)
    xT = np.ascontiguousarray(x.T)
    x_b = np.ascontiguousarray(x.astype(BF16))
    rwT = np.ascontiguousarray(inputs["router_w"].astype(np.float32).T)
    ebias = np.ascontiguousarray(inputs["e_bias"].astype(np.float32).reshape(E, 1))
    idn = np.eye(128, dtype=np.float32)
    U = np.triu(np.ones((128, 128), np.float32))
    iotaR = np.tile(np.arange(128, dtype=np.float32), (128, 1))
    tokid = (np.arange(128, dtype=np.float32)[:, None]
             + 128.0 * np.arange(NT, dtype=np.float32)[None, :])
    iotaW = np.ascontiguousarray(
        (np.arange(16, dtype=np.float32)[:, None]
         + 16.0 * np.arange(CAP // 16, dtype=np.float32)[None, :]))

    gup = inputs["gate_up_proj"].astype(np.float32)   # [E, 2I, H]
    dwp = inputs["down_proj"].astype(np.float32)      # [E, H, I]
    sgw = inputs["shared_gate_w"].astype(np.float32)  # [IS, H]
    suw = inputs["shared_up_w"].astype(np.float32)
    sdw = inputs["shared_down_w"].astype(np.float32)  # [H, IS]

    in_maps = []
    for c in range(NCORES):
        guT = np.empty((EPC, NK, NJ, 128, 128), BF16)
        dT = np.empty((EPC, NI, NHC, 128, 512), BF16)
        mskc = np.zeros((EPC, 128, E), np.float32)
        for l in range(EPC):
            e = EPC * c + l
            g = gup[e].T.astype(BF16)                 # [H, 2I]
            guT[l] = g.reshape(NK, 128, NJ, 128).transpose(0, 2, 1, 3)
            d = dwp[e].T.astype(BF16)                 # [I, H]
            dT[l] = d.reshape(NI, 128, NHC, 512).transpose(0, 2, 1, 3)
            mskc[l, :, e] = 1.0
        s0 = ISC * c
        in_maps.append({
            "xT_f32": xT, "x_bf16": x_b, "rwT": rwT, "ebias": ebias,
            "guT_r": guT, "dT_r": dT,
            "sgT": np.ascontiguousarray(sgw[s0:s0 + ISC].T.astype(BF16)),
            "suT": np.ascontiguousarray(suw[s0:s0 + ISC].T.astype(BF16)),
            "sdT": np.ascontiguousarray(sdw[:, s0:s0 + ISC].T.astype(BF16)),
            "idn": idn, "U128": U, "iotaR": iotaR, "tokid": tokid,
            "iotaW": iotaW, "msk": mskc,
        })
    return in_maps


def kernel(**inputs):
    from concourse.bass_utils import run_bass_kernel_spmd
    if "nc" not in _CACHE:
        _CACHE["nc"] = _build()
    nc = _CACHE["nc"]
    in_maps = _host_prep(inputs)
    res = run_bass_kernel_spmd(nc, in_maps, list(range(NCORES)))
    _CACHE["last_results"] = res
    out = np.concatenate([res.results[c]["out"] for c in range(NCORES)], axis=0)
    return out.reshape(B, S, H).astype(np.float32)


# revision 10
# speedup vs baseline: 1.2377x; 1.2377x over previous
"""Trainium2 Bass kernel for nn_MoE_27041114095775 (moe_routing).

Expert-parallel MoE across 8 NeuronCores:
  - fp32 router + top-4 selection on every core (exact selection)
  - 2 experts per core (gate_up/down sharded on the expert axis)
  - shared-expert FFN tensor-parallel along its intermediate dim (96/core)
  - on-device token compaction (cumsum via triangular matmuls, batched
    per-tile offsets via one 32-column matmul pair) feeding a transposing
    dma_gather; SwiGLU in bf16 with fp32 PSUM accumulate; gated outputs
    dma_scatter_add'ed (bf16) into a [T,H] accumulator
  - bf16 ReduceScatter(add) combines the 8 partials; each core returns a
    256-token fp32 slice.

Self-contained: hardcodes all shapes; host side only shards/reformats
inputs and reassembles the output.
"""
import numpy as np
import ml_dtypes
from contextlib import ExitStack

BF16 = ml_dtypes.bfloat16

# ---- problem dims (hardcoded) ----
B, S, H = 1, 2048, 2048
E, I, IS = 16, 768, 768
TOPK = 4
T = B * S
NCORES = 8
EPC = E // NCORES          # experts per core = 2
ISC = IS // NCORES         # shared intermediate per core = 96
CAP = 768                  # per-expert token capacity (multiple of 128)
NT = T // 128              # 16 token tiles
NK = H // 128              # 16 contraction tiles
NTC = CAP // 128           # 6 capacity tiles
NI = I // 128              # 6 intermediate tiles (= gate/up pair count)
NHC = H // 512             # 4 hidden 512-chunks

_CACHE = {}


def _build():
    import concourse.bass as bass
    import concourse.tile as tile
    from concourse import bacc, mybir, library_config
    from concourse.expressions import smin, smax

    f32 = mybir.dt.float32
    bf16 = mybir.dt.bfloat16
    i16 = mybir.dt.int16
    i32 = mybir.dt.int32
    MM = mybir.AluOpType
    AF = mybir.ActivationFunctionType

    nc = bacc.Bacc("TRN2", target_bir_lowering=False, debug=False,
                   num_devices=NCORES)

    # ---- external inputs ----
    xT_f32 = nc.dram_tensor("xT_f32", [H, T], f32, kind="ExternalInput")
    x_bf16 = nc.dram_tensor("x_bf16", [T, H], bf16, kind="ExternalInput")
    rwT = nc.dram_tensor("rwT", [H, E], f32, kind="ExternalInput")
    ebias = nc.dram_tensor("ebias", [E, 1], f32, kind="ExternalInput")
    gu_s = nc.dram_tensor("gu_s", [EPC, NI, NK, 128, 256], bf16, kind="ExternalInput")
    dT_r = nc.dram_tensor("dT_r", [EPC, NI, NHC, 128, 512], bf16, kind="ExternalInput")
    sgT = nc.dram_tensor("sgT", [H, ISC], bf16, kind="ExternalInput")
    suT = nc.dram_tensor("suT", [H, ISC], bf16, kind="ExternalInput")
    sdT = nc.dram_tensor("sdT", [ISC, H], bf16, kind="ExternalInput")
    idn = nc.dram_tensor("idn", [128, 128], f32, kind="ExternalInput")
    U128 = nc.dram_tensor("U128", [128, 128], f32, kind="ExternalInput")
    SLc = nc.dram_tensor("SLc", [2 * NT, 2 * NT + 2], f32, kind="ExternalInput")
    iotaR = nc.dram_tensor("iotaR", [128, 128], f32, kind="ExternalInput")
    tokid = nc.dram_tensor("tokid", [128, NT], f32, kind="ExternalInput")
    iotaW = nc.dram_tensor("iotaW", [16, CAP // 16], f32, kind="ExternalInput")
    msk = nc.dram_tensor("msk", [EPC, 128, E], f32, kind="ExternalInput")

    # ---- outputs ----
    out_p = nc.dram_tensor("out", [T // NCORES, H], f32, kind="ExternalOutput")

    # ---- internal DRAM ----
    acc = nc.dram_tensor("acc", [T, H], bf16)
    rs_res = nc.dram_tensor("rs_res", [T // NCORES, H], bf16)
    lists_d = nc.dram_tensor("lists_d", [EPC, T + 256], f32)
    gats_d = nc.dram_tensor("gats_d", [EPC, T + 256], f32)
    cnt_d = nc.dram_tensor("cnt_d", [EPC, 1], f32)

    with tile.TileContext(nc) as tc:
        with ExitStack() as ctx:
            cpool = ctx.enter_context(tc.tile_pool(name="consts", bufs=1))
            xpool = ctx.enter_context(tc.tile_pool(name="xstream", bufs=6))
            x2pool = ctx.enter_context(tc.tile_pool(name="x2stream", bufs=6))
            rpool = ctx.enter_context(tc.tile_pool(name="routing", bufs=1))
            tpool = ctx.enter_context(tc.tile_pool(name="topk", bufs=4))
            wpool = ctx.enter_context(tc.tile_pool(name="wgu", bufs=8))
            dpool = ctx.enter_context(tc.tile_pool(name="wdn", bufs=6))
            apool = ctx.enter_context(tc.tile_pool(name="acts", bufs=2))
            gpool = ctx.enter_context(tc.tile_pool(name="gath", bufs=1))
            spool = ctx.enter_context(tc.tile_pool(name="stage", bufs=2))
            scpool = ctx.enter_context(tc.tile_pool(name="scst", bufs=6))
            bigpool = ctx.enter_context(tc.tile_pool(name="bigstage", bufs=2))

            nc.gpsimd.load_library(library_config.mlp)

            # ---- constants ----
            idn_sb = cpool.tile([128, 128], f32)
            nc.sync.dma_start(idn_sb[:], idn[:])
            U_sb = cpool.tile([128, 128], f32)
            nc.sync.dma_start(U_sb[:], U128[:])
            SL_sb = cpool.tile([2 * NT, 2 * NT + 2], f32)
            nc.sync.dma_start(SL_sb[:], SLc[:])
            iotaR_sb = cpool.tile([128, 128], f32)
            nc.sync.dma_start(iotaR_sb[:], iotaR[:])
            tokid_sb = cpool.tile([128, NT], f32)
            nc.sync.dma_start(tokid_sb[:], tokid[:])
            iotaW_sb = cpool.tile([16, CAP // 16], f32)
            nc.sync.dma_start(iotaW_sb[:], iotaW[:])
            ebias_sb = cpool.tile([E, 1], f32)
            nc.sync.dma_start(ebias_sb[:], ebias[:])
            msk_sb = cpool.tile([128, EPC, E], f32)
            nc.sync.dma_start(msk_sb[:], msk[:].rearrange("l p e -> p l e"))
            rw_sb = cpool.tile([128, NK, E], f32)
            nc.sync.dma_start(rw_sb[:], rwT[:].rearrange("(k p) e -> p k e", p=128))
            sg_sb = cpool.tile([128, NK, ISC], bf16)
            nc.sync.dma_start(sg_sb[:], sgT[:].rearrange("(k p) i -> p k i", p=128))
            su_sb = cpool.tile([128, NK, ISC], bf16)
            nc.sync.dma_start(su_sb[:], suT[:].rearrange("(k p) i -> p k i", p=128))
            sd_sb = cpool.tile([ISC, H], bf16)
            nc.sync.dma_start(sd_sb[:], sdT[:])

            # ---- P1: router fp32 ----
            scT = rpool.tile([E, T], f32)
            bT = rpool.tile([E, T], f32)
            pA_cm = tc.tile_pool(name="pA", bufs=2, space="PSUM")
            pA = pA_cm.__enter__()
            pT_cm = tc.tile_pool(name="pT", bufs=2, space="PSUM")
            pT = pT_cm.__enter__()
            for tcn in range(4):
                t0 = 512 * tcn
                ps_r = pA.tile([E, 512], f32, tag="ps_r")
                for k in range(NK):
                    xt = xpool.tile([128, 512], f32, tag="xt")
                    nc.sync.dma_start(xt[:], xT_f32[128 * k:128 * (k + 1), t0:t0 + 512])
                    nc.tensor.matmul(ps_r[:], rw_sb[:, k, :], xt[:],
                                     start=(k == 0), stop=(k == NK - 1))
                nc.scalar.activation(scT[:, t0:t0 + 512], ps_r[:], AF.Sigmoid)
                nc.vector.tensor_scalar(out=bT[:, t0:t0 + 512], in0=scT[:, t0:t0 + 512],
                                        scalar1=ebias_sb[:, 0:1], scalar2=None,
                                        op0=MM.add)

            # ---- P2a: transpose scores/biased to [t, e] (PE, right after router) ----
            sb_all = rpool.tile([128, NT, 2 * E], f32)
            for j in range(NT):
                c0 = 128 * j
                tp_s = pT.tile([128, E], f32, tag="tpx", name=f"tp_s{j}")
                nc.tensor.transpose(tp_s[:], scT[:, c0:c0 + 128], idn_sb[0:E, 0:E])
                nc.vector.tensor_copy(sb_all[:, j, 0:E], tp_s[:])
                tp_b = pT.tile([128, E], f32, tag="tpx", name=f"tp_b{j}")
                nc.tensor.transpose(tp_b[:], bT[:, c0:c0 + 128], idn_sb[0:E, 0:E])
                nc.vector.tensor_copy(sb_all[:, j, E:2 * E], tp_b[:])

            # ---- P6 (emitted early so PE stays busy during routing DVE work):
            #      shared expert gate/up + down, initializes acc (bf16) ----
            s_act = rpool.tile([ISC, T], bf16)
            pS_cm = tc.tile_pool(name="pS", bufs=2, space="PSUM")
            pS = pS_cm.__enter__()
            for tcn in range(4):
                t0 = 512 * tcn
                ps_g = pS.tile([ISC, 512], f32, tag="ps_gu", name=f"ps_g{tcn}")
                ps_u = pS.tile([ISC, 512], f32, tag="ps_gu", name=f"ps_u{tcn}")
                for k in range(NK):
                    xt2 = x2pool.tile([128, 512], f32, tag="xt2")
                    nc.sync.dma_start(xt2[:], xT_f32[128 * k:128 * (k + 1), t0:t0 + 512])
                    xtb = x2pool.tile([128, 512], bf16, tag="xtb")
                    nc.vector.tensor_copy(xtb[:], xt2[:])
                    nc.tensor.matmul(ps_g[:], sg_sb[:, k, :], xtb[:],
                                     start=(k == 0), stop=(k == NK - 1))
                    nc.tensor.matmul(ps_u[:], su_sb[:, k, :], xtb[:],
                                     start=(k == 0), stop=(k == NK - 1))
                st = spool.tile([ISC, 512], f32, tag="silu_s")
                nc.scalar.activation(st[:], ps_g[:], AF.Silu)
                nc.vector.tensor_tensor(out=s_act[:, t0:t0 + 512], in0=st[:],
                                        in1=ps_u[:], op=MM.mult)
            for tt in range(NT):
                sto = bigpool.tile([128, H], bf16, tag="sto")
                for hc in range(NHC):
                    ps_o = pS.tile([128, 512], f32, tag="ps_o")
                    nc.tensor.matmul(ps_o[:], s_act[:, 128 * tt:128 * (tt + 1)],
                                     sd_sb[:, 512 * hc:512 * (hc + 1)],
                                     start=True, stop=True)
                    nc.vector.tensor_copy(sto[:, 512 * hc:512 * (hc + 1)], ps_o[:])
                nc.sync.dma_start(acc[128 * tt:128 * (tt + 1), :], sto[:])

            # ---- P2b: per-tile top-k + combine -> sel_all / per-tile staging ----
            pS_cm.__exit__(None, None, None)
            pB_cm = tc.tile_pool(name="pB", bufs=2, space="PSUM")
            pB = pB_cm.__enter__()
            sel_all = rpool.tile([128, 2 * NT], f32)
            st_all = rpool.tile([128, NT, EPC, 2], f32)
            pos_all = rpool.tile([128, NT, EPC], f32)
            for j in range(NT):
                sc_t = sb_all[:, j, 0:E]
                b_t = sb_all[:, j, E:2 * E]
                mx8 = tpool.tile([128, 8], f32, tag="mx8")
                nc.vector.max(out=mx8[:], in_=b_t[:])
                sel = tpool.tile([128, E], f32, tag="sel")
                nc.vector.tensor_scalar(out=sel[:], in0=b_t[:],
                                        scalar1=mx8[:, TOPK - 1:TOPK], scalar2=None,
                                        op0=MM.is_ge)
                w = tpool.tile([128, E], f32, tag="w")
                nc.vector.tensor_tensor(out=w[:], in0=sc_t[:], in1=sel[:], op=MM.mult)
                sums = tpool.tile([128, 1], f32, tag="sums")
                nc.vector.tensor_reduce(out=sums[:], in_=w[:],
                                        axis=mybir.AxisListType.X, op=MM.add)
                den = tpool.tile([128, 1], f32, tag="den")
                nc.vector.tensor_scalar(out=den[:], in0=sums[:], scalar1=1e-20,
                                        scalar2=None, op0=MM.add)
                rcp = tpool.tile([128, 1], f32, tag="rcp")
                nc.vector.reciprocal(rcp[:], den[:])
                comb = tpool.tile([128, E], f32, tag="comb")
                nc.vector.tensor_scalar(out=comb[:], in0=w[:], scalar1=rcp[:, 0:1],
                                        scalar2=None, op0=MM.mult)
                for l in range(EPC):
                    t16 = tpool.tile([128, E], f32, tag="t16")
                    nc.vector.tensor_tensor(out=t16[:], in0=sel[:],
                                            in1=msk_sb[:, l, :], op=MM.mult)
                    nc.vector.tensor_reduce(out=sel_all[:, 2 * j + l:2 * j + l + 1],
                                            in_=t16[:], axis=mybir.AxisListType.X,
                                            op=MM.add)
                    nc.vector.tensor_copy(st_all[:, j, l, 0:1], tokid_sb[:, j:j + 1])
                    t16b = tpool.tile([128, E], f32, tag="t16b")
                    nc.vector.tensor_tensor(out=t16b[:], in0=comb[:],
                                            in1=msk_sb[:, l, :], op=MM.mult)
                    nc.vector.tensor_reduce(out=st_all[:, j, l, 1:2], in_=t16b[:],
                                            axis=mybir.AxisListType.X, op=MM.add)
                # local inclusive cumsum -> exclusive positions (per tile)
                ps_pos = pB.tile([128, EPC], f32, tag="ps_small", name=f"ps_pos{j}")
                nc.tensor.matmul(ps_pos[:], U_sb[:],
                                 sel_all[:, 2 * j:2 * j + 2], start=True, stop=True)
                nc.vector.tensor_tensor(out=pos_all[:, j, :], in0=ps_pos[:],
                                        in1=sel_all[:, 2 * j:2 * j + 2],
                                        op=MM.subtract)

            # ---- P3: batched offsets: totals [32,1] then excl-prefix [34,1] ----
            ps_tot = pB.tile([2 * NT, 1], f32, tag="ps_small", name="ps_tot")
            nc.tensor.matmul(ps_tot[:], sel_all[:], U_sb[:, 127:128],
                             start=True, stop=True)
            tot_sb = rpool.tile([2 * NT, 1], f32)
            nc.vector.tensor_copy(tot_sb[:], ps_tot[:])
            ps_offs = pB.tile([2 * NT + 2, 1], f32, tag="ps_small", name="ps_offs")
            nc.tensor.matmul(ps_offs[:], SL_sb[:], tot_sb[:], start=True, stop=True)
            off_all = rpool.tile([2 * NT + 2, 1], f32)
            nc.vector.tensor_copy(off_all[:], ps_offs[:])
            off_i = rpool.tile([2 * NT + 2, 1], i32)
            nc.vector.tensor_copy(off_i[:], off_all[:])
            nc.sync.dma_start(cnt_d[:], off_all[2 * NT:2 * NT + 2, 0:1])

            # ---- P4: per-(tile, expert) segment build + scatter to DRAM ----
            for j in range(NT):
                for l in range(EPC):
                    O_l = tpool.tile([128, 128], f32, tag="O_l")
                    nc.vector.tensor_scalar(out=O_l[:], in0=iotaR_sb[:],
                                            scalar1=pos_all[:, j, l:l + 1],
                                            scalar2=sel_all[:, 2 * j + l:2 * j + l + 1],
                                            op0=MM.is_equal, op1=MM.mult)
                    ps_seg = pB.tile([128, 2], f32, tag="ps_small", name=f"ps_seg{j}_{l}")
                    nc.tensor.matmul(ps_seg[:], O_l[:], st_all[:, j, l, :],
                                     start=True, stop=True)
                    seg = tpool.tile([128, 2], f32, tag="seg")
                    nc.vector.tensor_copy(seg[:], ps_seg[:])
                    offv = nc.sync.value_load(off_i[2 * j + l:2 * j + l + 1, 0:1])
                    nc.sync.dma_start(lists_d[l, bass.ds(offv, 128)], seg[:, 0:1])
                    nc.sync.dma_start(gats_d[l, bass.ds(offv, 128)], seg[:, 1:2])
            pB_cm.__exit__(None, None, None)
            pT_cm.__exit__(None, None, None)
            pA_cm.__exit__(None, None, None)

            # ---- P5 + P7: per-expert gather, FFN, scatter ----
            pC_gu_cm = tc.tile_pool(name="pC_gu", bufs=2, space="PSUM")
            pC_gu = pC_gu_cm.__enter__()
            pC_y_cm = tc.tile_pool(name="pC_y", bufs=1, space="PSUM")
            pC_y = pC_y_cm.__enter__()
            for l in range(EPC):
                cntv = nc.gpsimd.value_load(off_i[2 * NT + l:2 * NT + l + 1, 0:1])
                cntc = smin(cntv, CAP)
                cnt16 = tpool.tile([16, 1], f32, tag="cnt16")
                nc.sync.dma_start(cnt16[:], cnt_d[l:l + 1, 0:1].to_broadcast([16, 1]))
                lw = tpool.tile([16, CAP // 16], f32, tag="lw")
                nc.sync.dma_start(
                    lw[:], lists_d[l, 0:CAP].rearrange("(f p) -> p f", p=16))
                m = tpool.tile([16, CAP // 16], f32, tag="m")
                nc.vector.tensor_scalar(out=m[:], in0=iotaW_sb[:],
                                        scalar1=cnt16[:, 0:1], scalar2=None,
                                        op0=MM.is_lt)
                t1 = tpool.tile([16, CAP // 16], f32, tag="t1")
                nc.vector.tensor_scalar(out=t1[:], in0=lw[:], scalar1=1.0,
                                        scalar2=None, op0=MM.add)
                nc.vector.tensor_tensor(out=t1[:], in0=t1[:], in1=m[:], op=MM.mult)
                nc.vector.tensor_scalar(out=t1[:], in0=t1[:], scalar1=1.0,
                                        scalar2=None, op0=MM.subtract)
                li = tpool.tile([16, CAP // 16], i16, tag="li")
                nc.vector.tensor_copy(li[:], t1[:])
                idx_l = gpool.tile([128, CAP // 16], i16, tag=f"idx{l}")
                nc.sync.dma_start(idx_l[0:16, :], li[:])
                nc.sync.dma_start(idx_l[16:32, :], idx_l[0:16, :])
                nc.sync.dma_start(idx_l[32:64, :], idx_l[0:32, :])
                nc.sync.dma_start(idx_l[64:128, :], idx_l[0:64, :])
                gat_l = gpool.tile([128, NTC], f32, tag=f"gat{l}")
                nc.sync.dma_start(
                    gat_l[:], gats_d[l, 0:CAP].rearrange("(f p) -> p f", p=128))

                xg = gpool.tile([128, NK, CAP], bf16, tag=f"xg{l}")
                nc.gpsimd.dma_gather(
                    out_ap=xg[:], in_ap=x_bf16[:], idxs_ap=idx_l[:],
                    num_idxs=CAP, num_idxs_reg=cntc, elem_size=H, transpose=True)

                # gate_up + SwiGLU (pair slabs: [128, 256] = gate | up)
                act_l = apool.tile([128, NI, CAP], bf16, tag="act")
                for pp in range(NI):
                    ps_gt = pC_gu.tile([128, CAP], f32, tag="ps_gu",
                                       name=f"ps_gt_{l}_{pp}")
                    ps_up = pC_gu.tile([128, CAP], f32, tag="ps_gu",
                                       name=f"ps_up_{l}_{pp}")
                    for k in range(NK):
                        wt = wpool.tile([128, 256], bf16, tag="wt_gu")
                        nc.sync.dma_start(wt[:], gu_s[l, pp, k, :, :])
                        nc.tensor.matmul(ps_gt[:, 0:512], wt[:, 0:128],
                                         xg[:, k, 0:512],
                                         start=(k == 0), stop=(k == NK - 1))
                        nc.tensor.matmul(ps_gt[:, 512:CAP], wt[:, 0:128],
                                         xg[:, k, 512:CAP],
                                         start=(k == 0), stop=(k == NK - 1))
                        nc.tensor.matmul(ps_up[:, 0:512], wt[:, 128:256],
                                         xg[:, k, 0:512],
                                         start=(k == 0), stop=(k == NK - 1))
                        nc.tensor.matmul(ps_up[:, 512:CAP], wt[:, 128:256],
                                         xg[:, k, 512:CAP],
                                         start=(k == 0), stop=(k == NK - 1))
                    silu_t = spool.tile([128, CAP], f32, tag="silu_t")
                    nc.scalar.activation(silu_t[:], ps_gt[:], AF.Silu)
                    nc.vector.tensor_tensor(out=act_l[:, pp, :], in0=silu_t[:],
                                            in1=ps_up[:], op=MM.mult)

                # down projection + gating + scatter-add (bf16)
                for half in range(2):
                    tts = [3 * half, 3 * half + 1, 3 * half + 2]
                    for hc in range(NHC):
                        ps_y = [pC_y.tile([128, 512], f32, tag=f"ps_y{q}",
                                          name=f"ps_y{q}_{l}_{half}_{hc}")
                                for q in range(3)]
                        for it in range(NI):
                            dwt = dpool.tile([128, 512], bf16, tag="wt_d")
                            nc.sync.dma_start(dwt[:], dT_r[l, it, hc, :, :])
                            for q, tt in enumerate(tts):
                                nc.tensor.matmul(
                                    ps_y[q][:],
                                    act_l[:, it, 128 * tt:128 * (tt + 1)], dwt[:],
                                    start=(it == 0), stop=(it == NI - 1))
                        for q, tt in enumerate(tts):
                            sc = scpool.tile([128, 1, 512], bf16, tag="sc")
                            nc.vector.tensor_scalar(out=sc[:, 0, :], in0=ps_y[q][:],
                                                    scalar1=gat_l[:, tt:tt + 1],
                                                    scalar2=None, op0=MM.mult)
                            ch = smin(smax(cntv - 128 * tt, 0), 128)
                            nc.gpsimd.dma_scatter_add(
                                out_ap=acc[:, 512 * hc:512 * (hc + 1)],
                                in_ap=sc[:], idxs_ap=idx_l[:, 8 * tt:8 * (tt + 1)],
                                num_idxs=128, num_idxs_reg=ch,
                                elem_size=512, elem_step=H)
            pC_y_cm.__exit__(None, None, None)
            pC_gu_cm.__exit__(None, None, None)

            # ---- P8: ReduceScatter (bf16) + fp32 output ----
            nc.gpsimd.collective_compute(
                "ReduceScatter", mybir.AluOpType.add,
                replica_groups=[list(range(NCORES))],
                ins=[acc[:]], outs=[rs_res[:]])
            for i in range(2):
                ot = bigpool.tile([128, H], bf16, tag="ot", name=f"ot{i}")
                nc.sync.dma_start(ot[:], rs_res[128 * i:128 * (i + 1), :])
                otf = bigpool.tile([128, H], f32, tag="otf", name=f"otf{i}")
                nc.vector.tensor_copy(otf[:], ot[:])
                nc.sync.dma_start(out_p[128 * i:128 * (i + 1), :], otf[:])

    nc.compile()
    return nc


def _host_prep(inputs):
    """Build the 8 per-core input maps from full inputs."""
    x = np.ascontiguousarray(inputs["hidden_states"].reshape(T, H), np.float32)
    xT = np.ascontiguousarray(x.T)
    x_b = np.ascontiguousarray(x.astype(BF16))
    rwT = np.ascontiguousarray(inputs["router_w"].astype(np.float32).T)
    ebias = np.ascontiguousarray(inputs["e_bias"].astype(np.float32).reshape(E, 1))
    idn = np.eye(128, dtype=np.float32)
    U = np.triu(np.ones((128, 128), np.float32))
    iotaR = np.tile(np.arange(128, dtype=np.float32), (128, 1))
    tokid = (np.arange(128, dtype=np.float32)[:, None]
             + 128.0 * np.arange(NT, dtype=np.float32)[None, :])
    iotaW = np.ascontiguousarray(
        (np.arange(16, dtype=np.float32)[:, None]
         + 16.0 * np.arange(CAP // 16, dtype=np.float32)[None, :]))
    # SL[c', c]: strict-lower (same expert) prefix matrix + count columns
    SL = np.zeros((2 * NT, 2 * NT + 2), np.float32)
    for cp in range(2 * NT):
        lp = cp % 2
        for c in range(2 * NT):
            if c % 2 == lp and cp // 2 < c // 2:
                SL[cp, c] = 1.0
        SL[cp, 2 * NT + lp] = 1.0

    gup = inputs["gate_up_proj"].astype(np.float32)   # [E, 2I, H]
    dwp = inputs["down_proj"].astype(np.float32)      # [E, H, I]
    sgw = inputs["shared_gate_w"].astype(np.float32)  # [IS, H]
    suw = inputs["shared_up_w"].astype(np.float32)
    sdw = inputs["shared_down_w"].astype(np.float32)  # [H, IS]

    in_maps = []
    for c in range(NCORES):
        gu_sl = np.empty((EPC, NI, NK, 128, 256), BF16)
        dT = np.empty((EPC, NI, NHC, 128, 512), BF16)
        mskc = np.zeros((EPC, 128, E), np.float32)
        for l in range(EPC):
            e = EPC * c + l
            g = gup[e].T.astype(BF16)                 # [H, 2I]
            r = g.reshape(NK, 128, 2 * NI, 128)       # [k, p, jj, j]
            pair = np.concatenate([r[:, :, 0:NI, :], r[:, :, NI:2 * NI, :]],
                                  axis=-1)            # [k, p, NI, 256]
            gu_sl[l] = pair.transpose(2, 0, 1, 3)     # [NI, k, p, 256]
            d = dwp[e].T.astype(BF16)                 # [I, H]
            dT[l] = d.reshape(NI, 128, NHC, 512).transpose(0, 2, 1, 3)
            mskc[l, :, e] = 1.0
        s0 = ISC * c
        in_maps.append({
            "xT_f32": xT, "x_bf16": x_b, "rwT": rwT, "ebias": ebias,
            "gu_s": gu_sl, "dT_r": dT,
            "sgT": np.ascontiguousarray(sgw[s0:s0 + ISC].T.astype(BF16)),
            "suT": np.ascontiguousarray(suw[s0:s0 + ISC].T.astype(BF16)),
            "sdT": np.ascontiguousarray(sdw[:, s0:s0 + ISC].T.astype(BF16)),
            "idn": idn, "U128": U, "SLc": SL, "iotaR": iotaR, "tokid": tokid,
            "iotaW": iotaW, "msk": mskc,
        })
    return in_maps


def kernel(**inputs):
    from concourse.bass_utils import run_bass_kernel_spmd
    if "nc" not in _CACHE:
        _CACHE["nc"] = _build()
    nc = _CACHE["nc"]
    in_maps = _host_prep(inputs)
    res = run_bass_kernel_spmd(nc, in_maps, list(range(NCORES)))
    _CACHE["last_results"] = res
    out = np.concatenate([res.results[c]["out"] for c in range(NCORES)], axis=0)
    return out.reshape(B, S, H).astype(np.float32)


# revision 11
# speedup vs baseline: 1.4179x; 1.1456x over previous
"""Trainium2 Bass kernel for nn_MoE_27041114095775 (moe_routing).

Expert-parallel MoE across 8 NeuronCores:
  - fp32 router + top-4 selection on every core (exact selection)
  - 2 experts per core (gate_up/down sharded on the expert axis)
  - shared-expert FFN tensor-parallel along its intermediate dim (96/core)
  - on-device token compaction (cumsum via triangular matmuls, batched
    per-tile offsets via one 32-column matmul pair) feeding a transposing
    dma_gather; SwiGLU in bf16 with fp32 PSUM accumulate; gated outputs
    dma_scatter_add'ed (bf16) into a [T,H] accumulator
  - bf16 ReduceScatter(add) combines the 8 partials; each core returns a
    256-token fp32 slice.

Self-contained: hardcodes all shapes; host side only shards/reformats
inputs and reassembles the output.
"""
import numpy as np
import ml_dtypes
from contextlib import ExitStack

BF16 = ml_dtypes.bfloat16

# ---- problem dims (hardcoded) ----
B, S, H = 1, 2048, 2048
E, I, IS = 16, 768, 768
TOPK = 4
T = B * S
NCORES = 8
EPC = E // NCORES          # experts per core = 2
ISC = IS // NCORES         # shared intermediate per core = 96
CAP = 768                  # per-expert token capacity (multiple of 128)
NT = T // 128              # 16 token tiles
NK = H // 128              # 16 contraction tiles
NTC = CAP // 128           # 6 capacity tiles
NI = I // 128              # 6 intermediate tiles (= gate/up pair count)
NHC = H // 512             # 4 hidden 512-chunks

_CACHE = {}


def _build():
    import concourse.bass as bass
    import concourse.tile as tile
    from concourse import bacc, mybir, library_config
    from concourse.expressions import smin, smax

    f32 = mybir.dt.float32
    bf16 = mybir.dt.bfloat16
    i16 = mybir.dt.int16
    i32 = mybir.dt.int32
    MM = mybir.AluOpType
    AF = mybir.ActivationFunctionType

    nc = bacc.Bacc("TRN2", target_bir_lowering=False, debug=False,
                   num_devices=NCORES)

    # ---- external inputs ----
    xT_f32 = nc.dram_tensor("xT_f32", [H, T], f32, kind="ExternalInput")
    x_bf16 = nc.dram_tensor("x_bf16", [T, H], bf16, kind="ExternalInput")
    rwT = nc.dram_tensor("rwT", [H, E], f32, kind="ExternalInput")
    ebias = nc.dram_tensor("ebias", [E, 1], f32, kind="ExternalInput")
    gu_s = nc.dram_tensor("gu_s", [EPC, NI, NK, 128, 256], bf16, kind="ExternalInput")
    dT_r = nc.dram_tensor("dT_r", [EPC, NI, NHC, 128, 512], bf16, kind="ExternalInput")
    sgT = nc.dram_tensor("sgT", [H, ISC], bf16, kind="ExternalInput")
    suT = nc.dram_tensor("suT", [H, ISC], bf16, kind="ExternalInput")
    sdT = nc.dram_tensor("sdT", [ISC, H], bf16, kind="ExternalInput")
    idn = nc.dram_tensor("idn", [128, 128], f32, kind="ExternalInput")
    U128 = nc.dram_tensor("U128", [128, 128], f32, kind="ExternalInput")
    SLc = nc.dram_tensor("SLc", [2 * NT, 2 * NT + 2], f32, kind="ExternalInput")
    iotaR = nc.dram_tensor("iotaR", [128, 128], f32, kind="ExternalInput")
    tokid = nc.dram_tensor("tokid", [128, NT], f32, kind="ExternalInput")
    iotaW = nc.dram_tensor("iotaW", [16, CAP // 16], f32, kind="ExternalInput")
    msk = nc.dram_tensor("msk", [EPC, 128, E], f32, kind="ExternalInput")

    # ---- outputs ----
    out_p = nc.dram_tensor("out", [T // NCORES, H], f32, kind="ExternalOutput")

    # ---- internal DRAM ----
    acc = nc.dram_tensor("acc", [T, H], bf16)
    rs_res = nc.dram_tensor("rs_res", [T // NCORES, H], bf16)
    lists_d = nc.dram_tensor("lists_d", [EPC, T + 256], f32)
    gats_d = nc.dram_tensor("gats_d", [EPC, T + 256], f32)
    cnt_d = nc.dram_tensor("cnt_d", [EPC, 1], f32)

    with tile.TileContext(nc) as tc:
        with ExitStack() as ctx:
            cpool = ctx.enter_context(tc.tile_pool(name="consts", bufs=1))
            xpool = ctx.enter_context(tc.tile_pool(name="xstream", bufs=6))
            x2pool = ctx.enter_context(tc.tile_pool(name="x2stream", bufs=6))
            rpool = ctx.enter_context(tc.tile_pool(name="routing", bufs=1))
            tpool = ctx.enter_context(tc.tile_pool(name="topk", bufs=4))
            wpool = ctx.enter_context(tc.tile_pool(name="wgu", bufs=8))
            dpool = ctx.enter_context(tc.tile_pool(name="wdn", bufs=6))
            apool = ctx.enter_context(tc.tile_pool(name="acts", bufs=2))
            gpool = ctx.enter_context(tc.tile_pool(name="gath", bufs=1))
            spool = ctx.enter_context(tc.tile_pool(name="stage", bufs=2))
            scpool = ctx.enter_context(tc.tile_pool(name="scst", bufs=6))
            bigpool = ctx.enter_context(tc.tile_pool(name="bigstage", bufs=2))

            nc.gpsimd.load_library(library_config.mlp)

            # ---- constants ----
            idn_sb = cpool.tile([128, 128], f32)
            nc.sync.dma_start(idn_sb[:], idn[:])
            U_sb = cpool.tile([128, 128], f32)
            nc.sync.dma_start(U_sb[:], U128[:])
            SL_sb = cpool.tile([2 * NT, 2 * NT + 2], f32)
            nc.sync.dma_start(SL_sb[:], SLc[:])
            iotaR_sb = cpool.tile([128, 128], f32)
            nc.sync.dma_start(iotaR_sb[:], iotaR[:])
            tokid_sb = cpool.tile([128, NT], f32)
            nc.sync.dma_start(tokid_sb[:], tokid[:])
            iotaW_sb = cpool.tile([16, CAP // 16], f32)
            nc.sync.dma_start(iotaW_sb[:], iotaW[:])
            ebias_sb = cpool.tile([E, 1], f32)
            nc.sync.dma_start(ebias_sb[:], ebias[:])
            msk_sb = cpool.tile([128, EPC, E], f32)
            nc.sync.dma_start(msk_sb[:], msk[:].rearrange("l p e -> p l e"))
            rw_sb = cpool.tile([128, NK, E], f32)
            nc.sync.dma_start(rw_sb[:], rwT[:].rearrange("(k p) e -> p k e", p=128))
            sg_sb = cpool.tile([128, NK, ISC], bf16)
            nc.sync.dma_start(sg_sb[:], sgT[:].rearrange("(k p) i -> p k i", p=128))
            su_sb = cpool.tile([128, NK, ISC], bf16)
            nc.sync.dma_start(su_sb[:], suT[:].rearrange("(k p) i -> p k i", p=128))
            sd_sb = cpool.tile([ISC, H], bf16)
            nc.sync.dma_start(sd_sb[:], sdT[:])

            # ---- P1: router fp32 ----
            scT = rpool.tile([E, T], f32)
            bT = rpool.tile([E, T], f32)
            pA_cm = tc.tile_pool(name="pA", bufs=2, space="PSUM")
            pA = pA_cm.__enter__()
            pT_cm = tc.tile_pool(name="pT", bufs=2, space="PSUM")
            pT = pT_cm.__enter__()
            for tcn in range(4):
                t0 = 512 * tcn
                ps_r = pA.tile([E, 512], f32, tag="ps_r")
                for k in range(NK):
                    xt = xpool.tile([128, 512], f32, tag="xt")
                    nc.sync.dma_start(xt[:], xT_f32[128 * k:128 * (k + 1), t0:t0 + 512])
                    nc.tensor.matmul(ps_r[:], rw_sb[:, k, :], xt[:],
                                     start=(k == 0), stop=(k == NK - 1))
                nc.scalar.activation(scT[:, t0:t0 + 512], ps_r[:], AF.Sigmoid)
                nc.vector.tensor_scalar(out=bT[:, t0:t0 + 512], in0=scT[:, t0:t0 + 512],
                                        scalar1=ebias_sb[:, 0:1], scalar2=None,
                                        op0=MM.add)

            # ---- P2a: transpose scores/biased to [t, e] (PE, right after router) ----
            sb_all = rpool.tile([128, NT, 2 * E], f32)
            for j in range(NT):
                c0 = 128 * j
                tp_s = pT.tile([128, E], f32, tag="tpx", name=f"tp_s{j}")
                nc.tensor.transpose(tp_s[:], scT[:, c0:c0 + 128], idn_sb[0:E, 0:E])
                nc.vector.tensor_copy(sb_all[:, j, 0:E], tp_s[:])
                tp_b = pT.tile([128, E], f32, tag="tpx", name=f"tp_b{j}")
                nc.tensor.transpose(tp_b[:], bT[:, c0:c0 + 128], idn_sb[0:E, 0:E])
                nc.vector.tensor_copy(sb_all[:, j, E:2 * E], tp_b[:])

            # ---- P6 (emitted early so PE stays busy during routing DVE work):
            #      shared expert gate/up + down, initializes acc (bf16) ----
            s_act = rpool.tile([ISC, T], bf16)
            pS_cm = tc.tile_pool(name="pS", bufs=2, space="PSUM")
            pS = pS_cm.__enter__()
            for tcn in range(4):
                t0 = 512 * tcn
                ps_g = pS.tile([ISC, 512], f32, tag="ps_gu", name=f"ps_g{tcn}")
                ps_u = pS.tile([ISC, 512], f32, tag="ps_gu", name=f"ps_u{tcn}")
                for k in range(NK):
                    xt2 = x2pool.tile([128, 512], f32, tag="xt2")
                    nc.sync.dma_start(xt2[:], xT_f32[128 * k:128 * (k + 1), t0:t0 + 512])
                    xtb = x2pool.tile([128, 512], bf16, tag="xtb")
                    nc.vector.tensor_copy(xtb[:], xt2[:])
                    nc.tensor.matmul(ps_g[:], sg_sb[:, k, :], xtb[:],
                                     start=(k == 0), stop=(k == NK - 1))
                    nc.tensor.matmul(ps_u[:], su_sb[:, k, :], xtb[:],
                                     start=(k == 0), stop=(k == NK - 1))
                st = spool.tile([ISC, 512], f32, tag="silu_s")
                nc.scalar.activation(st[:], ps_g[:], AF.Silu)
                nc.vector.tensor_tensor(out=s_act[:, t0:t0 + 512], in0=st[:],
                                        in1=ps_u[:], op=MM.mult)
            for tt in range(NT):
                sto = bigpool.tile([128, H], bf16, tag="sto")
                for hc in range(NHC):
                    ps_o = pS.tile([128, 512], f32, tag="ps_o")
                    nc.tensor.matmul(ps_o[:], s_act[:, 128 * tt:128 * (tt + 1)],
                                     sd_sb[:, 512 * hc:512 * (hc + 1)],
                                     start=True, stop=True)
                    nc.vector.tensor_copy(sto[:, 512 * hc:512 * (hc + 1)], ps_o[:])
                nc.sync.dma_start(acc[128 * tt:128 * (tt + 1), :], sto[:])

            # ---- P2b: per-tile top-k + combine -> sel_all / per-tile staging ----
            pS_cm.__exit__(None, None, None)
            pB_cm = tc.tile_pool(name="pB", bufs=2, space="PSUM")
            pB = pB_cm.__enter__()
            sel_all = rpool.tile([128, 2 * NT], f32)
            st_all = rpool.tile([128, NT, EPC, 2], f32)
            pos_all = rpool.tile([128, NT, EPC], f32)
            for j in range(NT):
                sc_t = sb_all[:, j, 0:E]
                b_t = sb_all[:, j, E:2 * E]
                mx8 = tpool.tile([128, 8], f32, tag="mx8")
                nc.vector.max(out=mx8[:], in_=b_t[:])
                sel = tpool.tile([128, E], f32, tag="sel")
                nc.vector.tensor_scalar(out=sel[:], in0=b_t[:],
                                        scalar1=mx8[:, TOPK - 1:TOPK], scalar2=None,
                                        op0=MM.is_ge)
                w = tpool.tile([128, E], f32, tag="w")
                nc.vector.tensor_tensor(out=w[:], in0=sc_t[:], in1=sel[:], op=MM.mult)
                sums = tpool.tile([128, 1], f32, tag="sums")
                nc.vector.tensor_reduce(out=sums[:], in_=w[:],
                                        axis=mybir.AxisListType.X, op=MM.add)
                den = tpool.tile([128, 1], f32, tag="den")
                nc.vector.tensor_scalar(out=den[:], in0=sums[:], scalar1=1e-20,
                                        scalar2=None, op0=MM.add)
                rcp = tpool.tile([128, 1], f32, tag="rcp")
                nc.vector.reciprocal(rcp[:], den[:])
                comb = tpool.tile([128, E], f32, tag="comb")
                nc.vector.tensor_scalar(out=comb[:], in0=w[:], scalar1=rcp[:, 0:1],
                                        scalar2=None, op0=MM.mult)
                for l in range(EPC):
                    t16 = tpool.tile([128, E], f32, tag="t16")
                    nc.vector.tensor_tensor(out=t16[:], in0=sel[:],
                                            in1=msk_sb[:, l, :], op=MM.mult)
                    nc.vector.tensor_reduce(out=sel_all[:, 2 * j + l:2 * j + l + 1],
                                            in_=t16[:], axis=mybir.AxisListType.X,
                                            op=MM.add)
                    nc.vector.tensor_copy(st_all[:, j, l, 0:1], tokid_sb[:, j:j + 1])
                    t16b = tpool.tile([128, E], f32, tag="t16b")
                    nc.vector.tensor_tensor(out=t16b[:], in0=comb[:],
                                            in1=msk_sb[:, l, :], op=MM.mult)
                    nc.vector.tensor_reduce(out=st_all[:, j, l, 1:2], in_=t16b[:],
                                            axis=mybir.AxisListType.X, op=MM.add)
                # local inclusive cumsum -> exclusive positions (per tile)
                ps_pos = pB.tile([128, EPC], f32, tag="ps_small", name=f"ps_pos{j}")
                nc.tensor.matmul(ps_pos[:], U_sb[:],
                                 sel_all[:, 2 * j:2 * j + 2], start=True, stop=True)
                nc.vector.tensor_tensor(out=pos_all[:, j, :], in0=ps_pos[:],
                                        in1=sel_all[:, 2 * j:2 * j + 2],
                                        op=MM.subtract)

            # ---- P3: batched offsets: totals [32,1] then excl-prefix [34,1] ----
            ps_tot = pB.tile([2 * NT, 1], f32, tag="ps_small", name="ps_tot")
            nc.tensor.matmul(ps_tot[:], sel_all[:], U_sb[:, 127:128],
                             start=True, stop=True)
            tot_sb = rpool.tile([2 * NT, 1], f32)
            nc.vector.tensor_copy(tot_sb[:], ps_tot[:])
            ps_offs = pB.tile([2 * NT + 2, 1], f32, tag="ps_small", name="ps_offs")
            nc.tensor.matmul(ps_offs[:], SL_sb[:], tot_sb[:], start=True, stop=True)
            off_all = rpool.tile([2 * NT + 2, 1], f32)
            nc.vector.tensor_copy(off_all[:], ps_offs[:])
            off_i = rpool.tile([2 * NT + 2, 1], i32)
            nc.vector.tensor_copy(off_i[:], off_all[:])
            nc.scalar.dma_start(cnt_d[:], off_all[2 * NT:2 * NT + 2, 0:1])

            # ---- P4: per-(tile, expert) segment build + scatter to DRAM ----
            for j in range(NT):
                for l in range(EPC):
                    O_l = tpool.tile([128, 128], f32, tag="O_l")
                    nc.vector.tensor_scalar(out=O_l[:], in0=iotaR_sb[:],
                                            scalar1=pos_all[:, j, l:l + 1],
                                            scalar2=sel_all[:, 2 * j + l:2 * j + l + 1],
                                            op0=MM.is_equal, op1=MM.mult)
                    ps_seg = pB.tile([128, 2], f32, tag="ps_small", name=f"ps_seg{j}_{l}")
                    nc.tensor.matmul(ps_seg[:], O_l[:], st_all[:, j, l, :],
                                     start=True, stop=True)
                    seg = tpool.tile([128, 2], f32, tag="seg")
                    nc.vector.tensor_copy(seg[:], ps_seg[:])
                    ps_sgT = pB.tile([2, 128], f32, tag="ps_small",
                                     name=f"ps_sgT{j}_{l}")
                    nc.tensor.transpose(ps_sgT[:], seg[:], idn_sb[:])
                    segT = tpool.tile([2, 128], f32, tag="segT")
                    nc.vector.tensor_copy(segT[:], ps_sgT[:])
                    offv = nc.scalar.value_load(off_i[2 * j + l:2 * j + l + 1, 0:1])
                    nc.scalar.dma_start(lists_d[l, bass.ds(offv, 128)], segT[0:1, :])
                    nc.scalar.dma_start(gats_d[l, bass.ds(offv, 128)], segT[1:2, :])
            pB_cm.__exit__(None, None, None)
            pT_cm.__exit__(None, None, None)
            pA_cm.__exit__(None, None, None)

            # ---- P5 + P7: per-expert gather, FFN, scatter ----
            pC_gu_cm = tc.tile_pool(name="pC_gu", bufs=3, space="PSUM")
            pC_gu = pC_gu_cm.__enter__()
            pC_y_cm = tc.tile_pool(name="pC_y", bufs=1, space="PSUM")
            pC_y = pC_y_cm.__enter__()
            for l in range(EPC):
                cntv = nc.gpsimd.value_load(off_i[2 * NT + l:2 * NT + l + 1, 0:1])
                cntc = smin(cntv, CAP)
                cnt16 = tpool.tile([16, 1], f32, tag="cnt16")
                nc.scalar.dma_start(cnt16[:], cnt_d[l:l + 1, 0:1].to_broadcast([16, 1]))
                lw = tpool.tile([16, CAP // 16], f32, tag="lw")
                nc.scalar.dma_start(
                    lw[:], lists_d[l, 0:CAP].rearrange("(f p) -> p f", p=16))
                m = tpool.tile([16, CAP // 16], f32, tag="m")
                nc.vector.tensor_scalar(out=m[:], in0=iotaW_sb[:],
                                        scalar1=cnt16[:, 0:1], scalar2=None,
                                        op0=MM.is_lt)
                t1 = tpool.tile([16, CAP // 16], f32, tag="t1")
                nc.vector.tensor_scalar(out=t1[:], in0=lw[:], scalar1=1.0,
                                        scalar2=None, op0=MM.add)
                nc.vector.tensor_tensor(out=t1[:], in0=t1[:], in1=m[:], op=MM.mult)
                nc.vector.tensor_scalar(out=t1[:], in0=t1[:], scalar1=1.0,
                                        scalar2=None, op0=MM.subtract)
                li = tpool.tile([16, CAP // 16], i16, tag="li")
                nc.vector.tensor_copy(li[:], t1[:])
                idx_l = gpool.tile([128, CAP // 16], i16, tag=f"idx{l}")
                nc.scalar.dma_start(idx_l[0:16, :], li[:])
                nc.scalar.dma_start(idx_l[16:32, :], idx_l[0:16, :])
                nc.scalar.dma_start(idx_l[32:64, :], idx_l[0:32, :])
                nc.scalar.dma_start(idx_l[64:128, :], idx_l[0:64, :])
                gat_l = gpool.tile([128, NTC], f32, tag=f"gat{l}")
                nc.scalar.dma_start(
                    gat_l[:], gats_d[l, 0:CAP].rearrange("(f p) -> p f", p=128))

                xg = gpool.tile([128, NK, CAP], bf16, tag=f"xg{l}")
                nc.gpsimd.dma_gather(
                    out_ap=xg[:], in_ap=x_bf16[:], idxs_ap=idx_l[:],
                    num_idxs=CAP, num_idxs_reg=cntc, elem_size=H, transpose=True)

                # gate_up + SwiGLU (pair slabs: [128, 256] = gate | up)
                act_l = apool.tile([128, NI, CAP], bf16, tag="act")
                for pp in range(NI):
                    ps_gt = pC_gu.tile([128, CAP], f32, tag="ps_gu",
                                       name=f"ps_gt_{l}_{pp}")
                    ps_up = pC_gu.tile([128, CAP], f32, tag="ps_gu",
                                       name=f"ps_up_{l}_{pp}")
                    for k in range(NK):
                        wt = wpool.tile([128, 256], bf16, tag="wt_gu")
                        nc.sync.dma_start(wt[:], gu_s[l, pp, k, :, :])
                        nc.tensor.matmul(ps_gt[:, 0:512], wt[:, 0:128],
                                         xg[:, k, 0:512],
                                         start=(k == 0), stop=(k == NK - 1))
                        nc.tensor.matmul(ps_gt[:, 512:CAP], wt[:, 0:128],
                                         xg[:, k, 512:CAP],
                                         start=(k == 0), stop=(k == NK - 1))
                        nc.tensor.matmul(ps_up[:, 0:512], wt[:, 128:256],
                                         xg[:, k, 0:512],
                                         start=(k == 0), stop=(k == NK - 1))
                        nc.tensor.matmul(ps_up[:, 512:CAP], wt[:, 128:256],
                                         xg[:, k, 512:CAP],
                                         start=(k == 0), stop=(k == NK - 1))
                    silu_t = spool.tile([128, CAP], bf16, tag="silu_t")
                    nc.scalar.activation(silu_t[:], ps_gt[:], AF.Silu)
                    nc.vector.tensor_tensor(out=act_l[:, pp, :], in0=silu_t[:],
                                            in1=ps_up[:], op=MM.mult)

                # down projection + gating + scatter-add (bf16)
                for half in range(3):
                    tts = [2 * half, 2 * half + 1]
                    for hc in range(NHC):
                        ps_y = [pC_y.tile([128, 512], f32, tag=f"ps_y{q}",
                                          name=f"ps_y{q}_{l}_{half}_{hc}")
                                for q in range(2)]
                        for it in range(NI):
                            dwt = dpool.tile([128, 512], bf16, tag="wt_d")
                            nc.sync.dma_start(dwt[:], dT_r[l, it, hc, :, :])
                            for q, tt in enumerate(tts):
                                nc.tensor.matmul(
                                    ps_y[q][:],
                                    act_l[:, it, 128 * tt:128 * (tt + 1)], dwt[:],
                                    start=(it == 0), stop=(it == NI - 1))
                        for q, tt in enumerate(tts):
                            sc = scpool.tile([128, 1, 512], bf16, tag="sc")
                            nc.vector.tensor_scalar(out=sc[:, 0, :], in0=ps_y[q][:],
                                                    scalar1=gat_l[:, tt:tt + 1],
                                                    scalar2=None, op0=MM.mult)
                            ch = smin(smax(cntv - 128 * tt, 0), 128)
                            nc.gpsimd.dma_scatter_add(
                                out_ap=acc[:, 512 * hc:512 * (hc + 1)],
                                in_ap=sc[:], idxs_ap=idx_l[:, 8 * tt:8 * (tt + 1)],
                                num_idxs=128, num_idxs_reg=ch,
                                elem_size=512, elem_step=H)
            pC_y_cm.__exit__(None, None, None)
            pC_gu_cm.__exit__(None, None, None)

            # ---- P8: ReduceScatter (bf16) + fp32 output ----
            nc.gpsimd.collective_compute(
                "ReduceScatter", mybir.AluOpType.add,
                replica_groups=[list(range(NCORES))],
                ins=[acc[:]], outs=[rs_res[:]])
            for i in range(2):
                ot = bigpool.tile([128, H], bf16, tag="ot", name=f"ot{i}")
                nc.sync.dma_start(ot[:], rs_res[128 * i:128 * (i + 1), :])
                otf = bigpool.tile([128, H], f32, tag="otf", name=f"otf{i}")
                nc.vector.tensor_copy(otf[:], ot[:])
                nc.sync.dma_start(out_p[128 * i:128 * (i + 1), :], otf[:])

    nc.compile()
    return nc


def _host_prep(inputs):
    """Build the 8 per-core input maps from full inputs."""
    x = np.ascontiguousarray(inputs["hidden_states"].reshape(T, H), np.float32)
    xT = np.ascontiguousarray(x.T)
    x_b = np.ascontiguousarray(x.astype(BF16))
    rwT = np.ascontiguousarray(inputs["router_w"].astype(np.float32).T)
    ebias = np.ascontiguousarray(inputs["e_bias"].astype(np.float32).reshape(E, 1))
    idn = np.eye(128, dtype=np.float32)
    U = np.triu(np.ones((128, 128), np.float32))
    iotaR = np.tile(np.arange(128, dtype=np.float32), (128, 1))
    tokid = (np.arange(128, dtype=np.float32)[:, None]
             + 128.0 * np.arange(NT, dtype=np.float32)[None, :])
    iotaW = np.ascontiguousarray(
        (np.arange(16, dtype=np.float32)[:, None]
         + 16.0 * np.arange(CAP // 16, dtype=np.float32)[None, :]))
    # SL[c', c]: strict-lower (same expert) prefix matrix + count columns
    SL = np.zeros((2 * NT, 2 * NT + 2), np.float32)
    for cp in range(2 * NT):
        lp = cp % 2
        for c in range(2 * NT):
            if c % 2 == lp and cp // 2 < c // 2:
                SL[cp, c] = 1.0
        SL[cp, 2 * NT + lp] = 1.0

    gup = inputs["gate_up_proj"].astype(np.float32)   # [E, 2I, H]
    dwp = inputs["down_proj"].astype(np.float32)      # [E, H, I]
    sgw = inputs["shared_gate_w"].astype(np.float32)  # [IS, H]
    suw = inputs["shared_up_w"].astype(np.float32)
    sdw = inputs["shared_down_w"].astype(np.float32)  # [H, IS]

    in_maps = []
    for c in range(NCORES):
        gu_sl = np.empty((EPC, NI, NK, 128, 256), BF16)
        dT = np.empty((EPC, NI, NHC, 128, 512), BF16)
        mskc = np.zeros((EPC, 128, E), np.float32)
        for l in range(EPC):
            e = EPC * c + l
            g = gup[e].T.astype(BF16)                 # [H, 2I]
            r = g.reshape(NK, 128, 2 * NI, 128)       # [k, p, jj, j]
            pair = np.concatenate([r[:, :, 0:NI, :], r[:, :, NI:2 * NI, :]],
                                  axis=-1)            # [k, p, NI, 256]
            gu_sl[l] = pair.transpose(2, 0, 1, 3)     # [NI, k, p, 256]
            d = dwp[e].T.astype(BF16)                 # [I, H]
            dT[l] = d.reshape(NI, 128, NHC, 512).transpose(0, 2, 1, 3)
            mskc[l, :, e] = 1.0
        s0 = ISC * c
        in_maps.append({
            "xT_f32": xT, "x_bf16": x_b, "rwT": rwT, "ebias": ebias,
            "gu_s": gu_sl, "dT_r": dT,
            "sgT": np.ascontiguousarray(sgw[s0:s0 + ISC].T.astype(BF16)),
            "suT": np.ascontiguousarray(suw[s0:s0 + ISC].T.astype(BF16)),
            "sdT": np.ascontiguousarray(sdw[:, s0:s0 + ISC].T.astype(BF16)),
            "idn": idn, "U128": U, "SLc": SL, "iotaR": iotaR, "tokid": tokid,
            "iotaW": iotaW, "msk": mskc,
        })
    return in_maps


def kernel(**inputs):
    from concourse.bass_utils import run_bass_kernel_spmd
    if "nc" not in _CACHE:
        _CACHE["nc"] = _build()
    nc = _CACHE["nc"]
    in_maps = _host_prep(inputs)
    res = run_bass_kernel_spmd(nc, in_maps, list(range(NCORES)))
    _CACHE["last_results"] = res
    out = np.concatenate([res.results[c]["out"] for c in range(NCORES)], axis=0)
    return out.reshape(B, S, H).astype(np.float32)
